# revision 20
# baseline (speedup 1.0000x reference)
"""AttentionPooling (ragged segment attention) on 8 Trainium2 NeuronCores.

Full inputs in, full output out. Strategy (data-parallel over graphs):
  - 128 graphs are load-balanced 16-per-core across 8 cores; each core gets
    its graphs' node embeddings (zero-padded to a multiple of 512 rows).
  - The single shared query is a model parameter, so the q-side is constant-
    folded on the host:  qk[h,e] = sum_d q_scaled[h,d]*k_w[h*64+d,e].
  - On device (per core), with cols c = h*16 + s (8 heads x 16 graph slots):
      scoresT[c, n] = sum_e qk_cols[e,c] * embT[e,n]          (PE, bf16)
      e[c, n]       = exp(scoresT + qb[c]) * indicator[c, n]  (ACT + DVE)
      e_cols        = PE-transpose(e)                          [n, c]
      pooled[c, :]  = sum_n e_cols[n,c] * emb[n,:]            (PE, accum)
      colsum[c]     = sum_n e_cols[n,c]  (+ host phantom correction)
      pooled       /= colsum                                  (DVE)
      o[s-block]    = blockdiag v-proj, then out-proj          (PE)
  - Host gathers the 8x[16,512] results back to [bs, 512].
"""

import numpy as np
import ml_dtypes

BF16 = ml_dtypes.bfloat16
FP8 = ml_dtypes.float8_e3m4
QK_SCALE = 128.0
E = 768
D = 512
H = 8
DH = 64
NCORES = 8
SLOTS = 16          # graphs per core
COLS = 128          # H * SLOTS
ES = E // 128       # 6 E-slices of 128

_prog_cache = {}


def _build_program(nc_pad):
    import concourse.bacc as bacc
    import concourse.tile as tile
    import concourse.mybir as mybir

    f32 = mybir.dt.float32
    bf16 = mybir.dt.bfloat16
    f8 = mybir.dt.float8e3
    AF = mybir.ActivationFunctionType

    nc = bacc.Bacc(None, target_bir_lowering=False)

    emb_d = nc.declare_dram_parameter("emb", [128, (nc_pad // 128) * E], bf16, isOutput=False)
    NGRP_ = nc_pad // 512
    embT_d = nc.declare_dram_parameter("embT", [NGRP_ * 128, ES * 512], f8, isOutput=False)
    ind_d = nc.declare_dram_parameter("mneg", [SLOTS, nc_pad], bf16, isOutput=False)
    ohB_d = nc.declare_dram_parameter("ohB", [SLOTS, COLS], bf16, isOutput=False)
    qk_d = nc.declare_dram_parameter("qk", [E, COLS], f8, isOutput=False)
    ph_d = nc.declare_dram_parameter("ph", [COLS, 1], f32, isOutput=False)
    vT_d = nc.declare_dram_parameter("vT", [E, D], bf16, isOutput=False)
    owT_d = nc.declare_dram_parameter("owT", [D, D], bf16, isOutput=False)
    ob_d = nc.declare_dram_parameter("ob", [SLOTS, D], f32, isOutput=False)
    id_d = nc.declare_dram_parameter("ident", [128, 128], bf16, isOutput=False)
    ones_d = nc.declare_dram_parameter("ones", [128, 1], bf16, isOutput=False)
    out_d = nc.declare_dram_parameter("out", [SLOTS, D], f32, isOutput=True)

    NGRP = nc_pad // 512         # 512-node groups
    NCH = nc_pad // 128          # 128-node chunks

    with tile.TileContext(nc) as tc:
        with (
            tc.tile_pool(name="const", bufs=1) as const,
            tc.tile_pool(name="embT_p", bufs=4) as embT_p,
            tc.tile_pool(name="emb_p", bufs=4) as emb_p,
            tc.tile_pool(name="e_p", bufs=4) as e_p,
            tc.tile_pool(name="ind_p", bufs=4) as ind_p,
            tc.tile_pool(name="ec_p", bufs=6) as ec_p,
            tc.tile_pool(name="small", bufs=1) as small,
            tc.tile_pool(name="psc", bufs=2, space="PSUM") as psc,
            tc.tile_pool(name="pst", bufs=2, space="PSUM") as pst,
            tc.tile_pool(name="pacc", bufs=1, space="PSUM") as pacc,
        ):
            # ---- critical-path constants first: qk, then group-0 embT ----
            qk_sb = const.tile([128, ES, COLS], f8)
            nc.sync.dma_start(out=qk_sb, in_=qk_d.rearrange("(s p) c -> p s c", p=128))
            ohB_sb = const.tile([SLOTS, COLS], bf16)
            nc.sync.dma_start(out=ohB_sb, in_=ohB_d[:, :])

            # ---- persistent accumulators (PSUM) ----
            ps_pool = pacc.tile([COLS, E], f32)      # pooled_u, 2 banks
            ps_cs = pacc.tile([COLS, 1], f32)        # col sums, 1 bank

            # embT is shipped pre-grouped: [NGRP, 128, ES, 512] with each
            # group's block contiguous in DRAM for clean big DMAs.
            embT_r = embT_d.rearrange("(g p) (s n) -> g p s n", p=128, n=512)

            def load_et(g):
                et = embT_p.tile([128, ES, 512], f8, tag="et")
                nc.sync.dma_start(out=et, in_=embT_r[g])
                mn_g = ind_p.tile([SLOTS, 512], bf16, tag="mn")
                nc.sync.dma_start(out=mn_g, in_=ind_d[:, g * 512:(g + 1) * 512])
                return et, mn_g

            def emit_scores(g, et_ind):
                et, mn_g = et_ind
                ps_s = psc.tile([COLS, 512], f32, tag="s")
                for s in range(ES):
                    nc.tensor.matmul(
                        ps_s, lhsT=qk_sb[:, s, :], rhs=et[:, s, :],
                        start=(s == 0), stop=False,
                    )
                # additive mask: += BIG*(ind-1); exp then zeroes non-members
                nc.tensor.matmul(ps_s, lhsT=ohB_sb, rhs=mn_g, start=False, stop=True)
                em_sb = e_p.tile([COLS, 512], bf16, tag="em")
                nc.scalar.activation(out=em_sb, in_=ps_s, func=AF.Exp, scale=1.0 / QK_SCALE)
                embt4 = emb_p.tile([128, 4, E], bf16)
                nc.sync.dma_start(
                    out=embt4,
                    in_=emb_d[:, g * 4 * E:(g + 1) * 4 * E].rearrange(
                        "p (j e) -> p j e", j=4))
                return em_sb, embt4

            def emit_pool(g, em_sb, embt4):
                for j in range(4):
                    ch = g * 4 + j
                    # e_cols chunk: [128 nodes, 128 cols] via PE transpose
                    ps_t = pst.tile([128, 128], bf16, tag="tr")
                    nc.tensor.transpose(ps_t, em_sb[:, j * 128:(j + 1) * 128], id_sb)
                    ec = ec_p.tile([128, COLS], bf16)
                    nc.vector.tensor_copy(ec, ps_t)

                    st = (ch == 0)
                    sp = (ch == NCH - 1)
                    nc.tensor.matmul(ps_pool[:, 0:512], lhsT=ec, rhs=embt4[:, j, 0:512],
                                     start=st, stop=sp)
                    nc.tensor.matmul(ps_pool[:, 512:768], lhsT=ec,
                                     rhs=embt4[:, j, 512:768], start=st, stop=sp)
                    nc.tensor.matmul(ps_cs, lhsT=ec, rhs=ones_sb, start=st, stop=sp)

            # Software pipeline: PE stays busy on group g+1's scores while
            # ACT/DVE produce group g's masked-exp, whose transposes+pools
            # are emitted (in PE program order) after those scores.
            ets = {0: load_et(0)}

            # secondary constants (needed a bit later than qk/et0)
            id_sb = const.tile([128, 128], bf16)
            nc.sync.dma_start(out=id_sb, in_=id_d[:, :])
            ones_sb = const.tile([128, 1], bf16)
            nc.sync.dma_start(out=ones_sb, in_=ones_d[:, :])

            pend = []
            for g in range(NGRP):
                em, embts = emit_scores(g, ets.pop(g))
                if g + 1 < NGRP:
                    ets[g + 1] = load_et(g + 1)
                pend.append((g, em, embts))
                if len(pend) > 2:
                    emit_pool(*pend.pop(0))
                if g == 1:
                    # tail-only constants: issue mid-stream so they're
                    # resident before the tail without delaying group 0/1
                    ph_sb = const.tile([COLS, 1], f32)
                    nc.sync.dma_start(out=ph_sb, in_=ph_d[:, :])
                    vT_sb = const.tile([128, ES, D], bf16)
                    nc.sync.dma_start(
                        out=vT_sb, in_=vT_d.rearrange("(s p) c -> p s c", p=128))
                    owT_sb = const.tile([128, 4, D], bf16)
                    nc.sync.dma_start(
                        out=owT_sb, in_=owT_d.rearrange("(s p) c -> p s c", p=128))
                    ob_sb = const.tile([SLOTS, D], f32)
                    nc.sync.dma_start(out=ob_sb, in_=ob_d[:, :])
            for p in pend:
                emit_pool(*p)

            # ---- normalize ----
            cs_sb = small.tile([COLS, 1], f32)
            nc.vector.tensor_add(cs_sb, ps_cs, ph_sb)
            rec_sb = small.tile([COLS, 1], f32)
            nc.vector.reciprocal(rec_sb, cs_sb)
            pooled = small.tile([COLS, E], bf16)
            nc.vector.tensor_scalar_mul(pooled, in0=ps_pool, scalar1=rec_sb)

            # ---- pooledT via PE transposes into one PSUM tile, one copy ----
            pT = small.tile([128, ES, COLS], bf16)
            ps_t2 = pacc.tile([128, ES, 128], bf16, tag="tail")
            for s in range(ES):
                nc.tensor.transpose(ps_t2[:, s, :], pooled[:, s * 128:(s + 1) * 128],
                                    id_sb)
            nc.vector.tensor_copy(pT, ps_t2)

            # ---- v-projection, directly in transposed layout ----
            # oT[i=(h*64+d), g] = sum_e vT[e, i] * pooledT[e, h*16+g]
            # i-chunk c4 covers heads {2c4, 2c4+1}; output lands at
            # partition base 64*(h%2) of oT_ps[:, c4, :].
            oT_ps = pacc.tile([128, 4, SLOTS], f32, tag="tail")
            for c4 in range(4):
                for hh in range(2):
                    h = 2 * c4 + hh
                    for s in range(ES):
                        nc.tensor.matmul(
                            oT_ps[64 * hh:64 * hh + 64, c4, :],
                            lhsT=vT_sb[:, s, h * DH:(h + 1) * DH],
                            rhs=pT[:, s, h * SLOTS:(h + 1) * SLOTS],
                            start=(s == 0), stop=(s == ES - 1),
                            tile_position=(0, 64 * hh),
                        )
            oT = small.tile([128, 4, SLOTS], bf16)
            nc.vector.tensor_copy(oT, oT_ps)

            # ---- out-projection: out[g, j] = sum_i o[g, i] * out_w[j, i] ----
            ps_f = psc.tile([SLOTS, D], f32, tag="s")
            for s in range(4):
                nc.tensor.matmul(ps_f, lhsT=oT[:, s, :], rhs=owT_sb[:, s, :],
                                 start=(s == 0), stop=(s == 3))

            res = small.tile([SLOTS, D], f32)
            nc.vector.tensor_add(res, ps_f, ob_sb)
            nc.sync.dma_start(out=out_d[:, :], in_=res)

    nc.finalize()
    return nc


def _host_prep(graph_emb, qry, q_w, k_w, v_w, in_b, out_w, out_b, ptr, batch):
    graph_emb = np.asarray(graph_emb, dtype=np.float32)
    qry = np.asarray(qry, dtype=np.float32)
    q_w = np.asarray(q_w, dtype=np.float32)
    k_w = np.asarray(k_w, dtype=np.float32)
    v_w = np.asarray(v_w, dtype=np.float32)
    in_b = np.asarray(in_b, dtype=np.float32)
    out_w = np.asarray(out_w, dtype=np.float32)
    out_b = np.asarray(out_b, dtype=np.float32)
    ptr = np.asarray(ptr).astype(np.int64)
    batch = np.asarray(batch).astype(np.int64)

    N = graph_emb.shape[0]
    B = len(ptr) - 1
    assert B <= NCORES * SLOTS, f"too many graphs: {B}"
    assert int(batch.max()) < B, "batch id out of ptr range"
    n_nodes = ptr[1:] - ptr[:-1]
    max_node = int(n_nodes.max()) + 1
    bs = int(batch.max()) + 1

    # --- mirror the reference's scatter semantics (jnp .at[] wraps negatives,
    # drops OOB, last write wins; valid mask is by slot index) ---
    pos = np.arange(N) - ptr[batch]
    m = np.where(pos < 0, pos + max_node, pos)
    part = (m >= 0) & (m < max_node) & (m < n_nodes[batch])
    idx = np.nonzero(part)[0]
    key = batch[idx] * max_node + m[idx]
    _, first_rev = np.unique(key[::-1], return_index=True)
    keep = idx[::-1][first_rev]
    keep.sort()
    kb = batch[keep]
    counts = np.bincount(kb, minlength=B)
    phantom = n_nodes.astype(np.float64) - counts  # valid-but-unfilled slots

    # --- q-side constant folding (qry is a model parameter) ---
    bq, bk, bv = in_b[:D], in_b[D:2 * D], in_b[2 * D:]
    scale = DH ** -0.5
    q = ((qry.reshape(-1)[-D:] @ q_w.T) + bq) * scale
    qh = q.reshape(H, DH)
    qk = np.stack([qh[h] @ k_w[h * DH:(h + 1) * DH, :] for h in range(H)])  # [8, E]
    qb = np.einsum("hd,hd->h", qh, bk.reshape(H, DH))                        # [8]
    ob_eff = out_b + out_w @ bv

    # --- balanced assignment: 16 graphs per core, boustrophedon by size ---
    order = np.argsort(-counts, kind="stable")
    slot_of = np.empty(B, dtype=np.int64)   # graph -> core*16+slot
    for r in range(SLOTS):
        row = order[r * NCORES:(r + 1) * NCORES]
        seq = range(NCORES) if r % 2 == 0 else range(NCORES - 1, -1, -1)
        for c, gi in zip(seq, row):
            slot_of[gi] = c * SLOTS + r

    nodes_of = [[] for _ in range(B)]
    for n in keep:
        nodes_of[batch[n]].append(n)

    core_loads = np.zeros(NCORES, dtype=np.int64)
    for gi in range(B):
        core_loads[slot_of[gi] // SLOTS] += counts[gi]
    nc_pad = max(512, int(np.ceil(core_loads.max() / 512.0)) * 512)

    exp_qb = np.exp(qb)
    BIG = 30.0 * QK_SCALE
    ohB = np.zeros((SLOTS, COLS), dtype=BF16)
    for c in range(COLS):
        ohB[c % SLOTS, c] = BIG

    in_maps = []
    for c in range(NCORES):
        rows = []
        ind16 = np.zeros((SLOTS, nc_pad), dtype=np.float32)
        ph_col = np.zeros((COLS, 1), dtype=np.float32)
        off = 0
        for s in range(SLOTS):
            gis = np.nonzero(slot_of == c * SLOTS + s)[0]
            if len(gis) == 0:
                continue
            gi = int(gis[0])
            ns = nodes_of[gi]
            rows.extend(ns)
            ind16[s, off:off + len(ns)] = 1
            off += len(ns)
            for h in range(H):
                ph_col[h * SLOTS + s, 0] = phantom[gi]
        emb_c = np.zeros((nc_pad, E), dtype=BF16)
        if rows:
            emb_c[:len(rows)] = graph_emb[np.asarray(rows)].astype(BF16)
        in_maps.append({
            "emb": np.ascontiguousarray(
                emb_c.reshape(nc_pad // 128, 128, E).transpose(1, 0, 2)
                .reshape(128, (nc_pad // 128) * E)),
            "embT": np.ascontiguousarray(
                emb_c.reshape(nc_pad // 512, 512, ES, 128)
                .transpose(0, 3, 2, 1)
                .reshape(nc_pad // 512 * 128, ES * 512)).astype(FP8),
            "mneg": np.ascontiguousarray(ind16 - 1.0).astype(BF16),
            "ohB": ohB,
            "qk": np.ascontiguousarray((np.repeat(qk, SLOTS, axis=0) * QK_SCALE).T.astype(FP8)),
            "ph": ph_col,
            "vT": np.ascontiguousarray(v_w.T).astype(BF16),
            "owT": np.ascontiguousarray(out_w.T).astype(BF16),
            "ob": np.broadcast_to(ob_eff, (SLOTS, D)).astype(np.float32).copy(),
            "ident": np.eye(128, dtype=BF16),
            "ones": np.ones((128, 1), dtype=BF16),
        })

    meta = {
        "bs": bs,
        "slot_of": slot_of,
        "n_nodes": n_nodes,
        "nc_pad": nc_pad,
    }
    return in_maps, meta


def _assemble(results, meta):
    bs = meta["bs"]
    slot_of = meta["slot_of"]
    n_nodes = meta["n_nodes"]
    out = np.empty((bs, D), dtype=np.float32)
    for b in range(bs):
        sl = int(slot_of[b])
        out[b] = results[sl // SLOTS]["out"][sl % SLOTS]
        if n_nodes[b] <= 0:
            out[b] = np.nan
    return out


def kernel(graph_emb, qry, q_w, k_w, v_w, in_b, out_w, out_b, ptr, batch):
    from concourse.bass_utils import run_bass_kernel_spmd

    in_maps, meta = _host_prep(graph_emb, qry, q_w, k_w, v_w, in_b, out_w,
                               out_b, ptr, batch)
    nc_pad = meta["nc_pad"]
    if nc_pad not in _prog_cache:
        _prog_cache[nc_pad] = _build_program(nc_pad)
    nc = _prog_cache[nc_pad]
    res = run_bass_kernel_spmd(nc, in_maps, list(range(NCORES)))
    return _assemble(res.results, meta)


# revision 21
# speedup vs baseline: 1.0151x; 1.0151x over previous
"""AttentionPooling (ragged segment attention) on 8 Trainium2 NeuronCores.

Full inputs in, full output out. Strategy (data-parallel over graphs):
  - 128 graphs are load-balanced 16-per-core across 8 cores; each core gets
    its graphs' node embeddings (zero-padded to a multiple of 512 rows).
  - The single shared query is a model parameter, so the q-side is constant-
    folded on the host:  qk[h,e] = sum_d q_scaled[h,d]*k_w[h*64+d,e].
  - On device (per core), with cols c = h*16 + s (8 heads x 16 graph slots):
      scoresT[c, n] = sum_e qk_cols[e,c] * embT[e,n]          (PE, bf16)
      e[c, n]       = exp(scoresT + qb[c]) * indicator[c, n]  (ACT + DVE)
      e_cols        = PE-transpose(e)                          [n, c]
      pooled[c, :]  = sum_n e_cols[n,c] * emb[n,:]            (PE, accum)
      colsum[c]     = sum_n e_cols[n,c]  (+ host phantom correction)
      pooled       /= colsum                                  (DVE)
      o[s-block]    = blockdiag v-proj, then out-proj          (PE)
  - Host gathers the 8x[16,512] results back to [bs, 512].
"""

import numpy as np
import ml_dtypes

BF16 = ml_dtypes.bfloat16
FP8 = ml_dtypes.float8_e3m4
QK_SCALE = 128.0
E = 768
D = 512
H = 8
DH = 64
NCORES = 8
SLOTS = 16          # graphs per core
COLS = 128          # H * SLOTS
ES = E // 128       # 6 E-slices of 128

_prog_cache = {}


def _build_program(nc_pad):
    import concourse.bacc as bacc
    import concourse.tile as tile
    import concourse.mybir as mybir

    f32 = mybir.dt.float32
    bf16 = mybir.dt.bfloat16
    f8 = mybir.dt.float8e3
    AF = mybir.ActivationFunctionType

    nc = bacc.Bacc(None, target_bir_lowering=False)

    emb_d = nc.declare_dram_parameter("emb", [128, (nc_pad // 128) * E], bf16, isOutput=False)
    NGRP_ = nc_pad // 512
    embT_d = nc.declare_dram_parameter("embT", [NGRP_ * 128, ES * 512], f8, isOutput=False)
    ind_d = nc.declare_dram_parameter("mneg", [SLOTS, nc_pad], bf16, isOutput=False)
    ohB_d = nc.declare_dram_parameter("ohB", [SLOTS, COLS], bf16, isOutput=False)
    qk_d = nc.declare_dram_parameter("qk", [E, COLS], f8, isOutput=False)
    ph_d = nc.declare_dram_parameter("ph", [COLS, 1], f32, isOutput=False)
    vT_d = nc.declare_dram_parameter("vT", [E, D], bf16, isOutput=False)
    owT_d = nc.declare_dram_parameter("owT", [D, D], bf16, isOutput=False)
    ob_d = nc.declare_dram_parameter("ob", [SLOTS, D], f32, isOutput=False)
    id_d = nc.declare_dram_parameter("ident", [128, 128], bf16, isOutput=False)
    ones_d = nc.declare_dram_parameter("ones", [128, 1], bf16, isOutput=False)
    out_d = nc.declare_dram_parameter("out", [SLOTS, D], f32, isOutput=True)

    NGRP = nc_pad // 512         # 512-node groups
    NCH = nc_pad // 128          # 128-node chunks

    with tile.TileContext(nc) as tc:
        with (
            tc.tile_pool(name="const", bufs=1) as const,
            tc.tile_pool(name="embT_p", bufs=4) as embT_p,
            tc.tile_pool(name="emb_p", bufs=4) as emb_p,
            tc.tile_pool(name="e_p", bufs=4) as e_p,
            tc.tile_pool(name="ind_p", bufs=4) as ind_p,
            tc.tile_pool(name="ec_p", bufs=6) as ec_p,
            tc.tile_pool(name="small", bufs=1) as small,
            tc.tile_pool(name="psc", bufs=2, space="PSUM") as psc,
            tc.tile_pool(name="pst", bufs=2, space="PSUM") as pst,
            tc.tile_pool(name="pacc", bufs=1, space="PSUM") as pacc,
        ):
            # ---- critical-path constants first: qk, then group-0 embT ----
            qk_sb = const.tile([128, ES, COLS], f8)
            nc.sync.dma_start(out=qk_sb, in_=qk_d.rearrange("(s p) c -> p s c", p=128))
            ohB_sb = const.tile([SLOTS, COLS], bf16)
            nc.sync.dma_start(out=ohB_sb, in_=ohB_d[:, :])

            # ---- persistent accumulators (PSUM) ----
            ps_pool = pacc.tile([COLS, E], f32)      # pooled_u, 2 banks
            ps_cs = pacc.tile([COLS, 1], f32)        # col sums, 1 bank

            # embT is shipped pre-grouped: [NGRP, 128, ES, 512] with each
            # group's block contiguous in DRAM for clean big DMAs.
            embT_r = embT_d.rearrange("(g p) (s n) -> g p s n", p=128, n=512)

            def load_et(g):
                ha = ES // 2
                et_a = embT_p.tile([128, ha, 512], f8, tag="eta")
                nc.sync.dma_start(out=et_a, in_=embT_r[g][:, 0:ha, :])
                et_b = embT_p.tile([128, ES - ha, 512], f8, tag="etb")
                nc.sync.dma_start(out=et_b, in_=embT_r[g][:, ha:ES, :])
                mn_g = ind_p.tile([SLOTS, 512], bf16, tag="mn")
                nc.sync.dma_start(out=mn_g, in_=ind_d[:, g * 512:(g + 1) * 512])
                return et_a, et_b, mn_g

            def emit_scores(g, et_ind):
                et_a, et_b, mn_g = et_ind
                ha = ES // 2
                ps_s = psc.tile([COLS, 512], f32, tag="s")
                for s in range(ES):
                    et_sl = et_a[:, s, :] if s < ha else et_b[:, s - ha, :]
                    nc.tensor.matmul(
                        ps_s, lhsT=qk_sb[:, s, :], rhs=et_sl,
                        start=(s == 0), stop=False,
                    )
                # additive mask: += BIG*(ind-1); exp then zeroes non-members
                nc.tensor.matmul(ps_s, lhsT=ohB_sb, rhs=mn_g, start=False, stop=True)
                em_sb = e_p.tile([COLS, 512], bf16, tag="em")
                nc.scalar.activation(out=em_sb, in_=ps_s, func=AF.Exp, scale=1.0 / QK_SCALE)
                embt4 = emb_p.tile([128, 4, E], bf16)
                nc.sync.dma_start(
                    out=embt4,
                    in_=emb_d[:, g * 4 * E:(g + 1) * 4 * E].rearrange(
                        "p (j e) -> p j e", j=4))
                return em_sb, embt4

            def emit_pool(g, em_sb, embt4):
                for j in range(4):
                    ch = g * 4 + j
                    # e_cols chunk: [128 nodes, 128 cols] via PE transpose
                    ps_t = pst.tile([128, 128], bf16, tag="tr")
                    nc.tensor.transpose(ps_t, em_sb[:, j * 128:(j + 1) * 128], id_sb)
                    ec = ec_p.tile([128, COLS], bf16)
                    nc.vector.tensor_copy(ec, ps_t)

                    st = (ch == 0)
                    sp = (ch == NCH - 1)
                    nc.tensor.matmul(ps_pool[:, 0:512], lhsT=ec, rhs=embt4[:, j, 0:512],
                                     start=st, stop=sp)
                    nc.tensor.matmul(ps_pool[:, 512:768], lhsT=ec,
                                     rhs=embt4[:, j, 512:768], start=st, stop=sp)
                    nc.tensor.matmul(ps_cs, lhsT=ec, rhs=ones_sb, start=st, stop=sp)

            # Software pipeline: PE stays busy on group g+1's scores while
            # ACT/DVE produce group g's masked-exp, whose transposes+pools
            # are emitted (in PE program order) after those scores.
            ets = {0: load_et(0)}

            # secondary constants (needed a bit later than qk/et0)
            id_sb = const.tile([128, 128], bf16)
            nc.sync.dma_start(out=id_sb, in_=id_d[:, :])
            ones_sb = const.tile([128, 1], bf16)
            nc.sync.dma_start(out=ones_sb, in_=ones_d[:, :])

            pend = []
            for g in range(NGRP):
                em, embts = emit_scores(g, ets.pop(g))
                if g + 1 < NGRP:
                    ets[g + 1] = load_et(g + 1)
                pend.append((g, em, embts))
                if len(pend) > 2:
                    emit_pool(*pend.pop(0))
                if g == 1:
                    # tail-only constants: issue mid-stream so they're
                    # resident before the tail without delaying group 0/1
                    ph_sb = const.tile([COLS, 1], f32)
                    nc.sync.dma_start(out=ph_sb, in_=ph_d[:, :])
                    vT_sb = const.tile([128, ES, D], bf16)
                    nc.sync.dma_start(
                        out=vT_sb, in_=vT_d.rearrange("(s p) c -> p s c", p=128))
                    owT_sb = const.tile([128, 4, D], bf16)
                    nc.sync.dma_start(
                        out=owT_sb, in_=owT_d.rearrange("(s p) c -> p s c", p=128))
                    ob_sb = const.tile([SLOTS, D], f32)
                    nc.sync.dma_start(out=ob_sb, in_=ob_d[:, :])
            for p in pend:
                emit_pool(*p)

            # ---- normalize ----
            cs_sb = small.tile([COLS, 1], f32)
            nc.vector.tensor_add(cs_sb, ps_cs, ph_sb)
            rec_sb = small.tile([COLS, 1], f32)
            nc.vector.reciprocal(rec_sb, cs_sb)
            pooled = small.tile([COLS, E], bf16)
            nc.vector.tensor_scalar_mul(pooled, in0=ps_pool, scalar1=rec_sb)

            # ---- pooledT via PE transposes into one PSUM tile, one copy ----
            pT = small.tile([128, ES, COLS], bf16)
            ps_t2 = pacc.tile([128, ES, 128], bf16, tag="tail")
            for s in range(ES):
                nc.tensor.transpose(ps_t2[:, s, :], pooled[:, s * 128:(s + 1) * 128],
                                    id_sb)
            nc.vector.tensor_copy(pT, ps_t2)

            # ---- v-projection, directly in transposed layout ----
            # oT[i=(h*64+d), g] = sum_e vT[e, i] * pooledT[e, h*16+g]
            # i-chunk c4 covers heads {2c4, 2c4+1}; output lands at
            # partition base 64*(h%2) of oT_ps[:, c4, :].
            oT_ps = pacc.tile([128, 4, SLOTS], f32, tag="tail")
            for c4 in range(4):
                for hh in range(2):
                    h = 2 * c4 + hh
                    for s in range(ES):
                        nc.tensor.matmul(
                            oT_ps[64 * hh:64 * hh + 64, c4, :],
                            lhsT=vT_sb[:, s, h * DH:(h + 1) * DH],
                            rhs=pT[:, s, h * SLOTS:(h + 1) * SLOTS],
                            start=(s == 0), stop=(s == ES - 1),
                            tile_position=(0, 64 * hh),
                        )
            oT = small.tile([128, 4, SLOTS], bf16)
            nc.vector.tensor_copy(oT, oT_ps)

            # ---- out-projection: out[g, j] = sum_i o[g, i] * out_w[j, i] ----
            ps_f = psc.tile([SLOTS, D], f32, tag="s")
            for s in range(4):
                nc.tensor.matmul(ps_f, lhsT=oT[:, s, :], rhs=owT_sb[:, s, :],
                                 start=(s == 0), stop=(s == 3))

            res = small.tile([SLOTS, D], f32)
            nc.vector.tensor_add(res, ps_f, ob_sb)
            nc.sync.dma_start(out=out_d[:, :], in_=res)

    nc.finalize()
    return nc


def _host_prep(graph_emb, qry, q_w, k_w, v_w, in_b, out_w, out_b, ptr, batch):
    graph_emb = np.asarray(graph_emb, dtype=np.float32)
    qry = np.asarray(qry, dtype=np.float32)
    q_w = np.asarray(q_w, dtype=np.float32)
    k_w = np.asarray(k_w, dtype=np.float32)
    v_w = np.asarray(v_w, dtype=np.float32)
    in_b = np.asarray(in_b, dtype=np.float32)
    out_w = np.asarray(out_w, dtype=np.float32)
    out_b = np.asarray(out_b, dtype=np.float32)
    ptr = np.asarray(ptr).astype(np.int64)
    batch = np.asarray(batch).astype(np.int64)

    N = graph_emb.shape[0]
    B = len(ptr) - 1
    assert B <= NCORES * SLOTS, f"too many graphs: {B}"
    assert int(batch.max()) < B, "batch id out of ptr range"
    n_nodes = ptr[1:] - ptr[:-1]
    max_node = int(n_nodes.max()) + 1
    bs = int(batch.max()) + 1

    # --- mirror the reference's scatter semantics (jnp .at[] wraps negatives,
    # drops OOB, last write wins; valid mask is by slot index) ---
    pos = np.arange(N) - ptr[batch]
    m = np.where(pos < 0, pos + max_node, pos)
    part = (m >= 0) & (m < max_node) & (m < n_nodes[batch])
    idx = np.nonzero(part)[0]
    key = batch[idx] * max_node + m[idx]
    _, first_rev = np.unique(key[::-1], return_index=True)
    keep = idx[::-1][first_rev]
    keep.sort()
    kb = batch[keep]
    counts = np.bincount(kb, minlength=B)
    phantom = n_nodes.astype(np.float64) - counts  # valid-but-unfilled slots

    # --- q-side constant folding (qry is a model parameter) ---
    bq, bk, bv = in_b[:D], in_b[D:2 * D], in_b[2 * D:]
    scale = DH ** -0.5
    q = ((qry.reshape(-1)[-D:] @ q_w.T) + bq) * scale
    qh = q.reshape(H, DH)
    qk = np.stack([qh[h] @ k_w[h * DH:(h + 1) * DH, :] for h in range(H)])  # [8, E]
    qb = np.einsum("hd,hd->h", qh, bk.reshape(H, DH))                        # [8]
    ob_eff = out_b + out_w @ bv

    # --- balanced assignment: 16 graphs per core, boustrophedon by size ---
    order = np.argsort(-counts, kind="stable")
    slot_of = np.empty(B, dtype=np.int64)   # graph -> core*16+slot
    for r in range(SLOTS):
        row = order[r * NCORES:(r + 1) * NCORES]
        seq = range(NCORES) if r % 2 == 0 else range(NCORES - 1, -1, -1)
        for c, gi in zip(seq, row):
            slot_of[gi] = c * SLOTS + r

    nodes_of = [[] for _ in range(B)]
    for n in keep:
        nodes_of[batch[n]].append(n)

    core_loads = np.zeros(NCORES, dtype=np.int64)
    for gi in range(B):
        core_loads[slot_of[gi] // SLOTS] += counts[gi]
    nc_pad = max(512, int(np.ceil(core_loads.max() / 512.0)) * 512)

    exp_qb = np.exp(qb)
    BIG = 30.0 * QK_SCALE
    ohB = np.zeros((SLOTS, COLS), dtype=BF16)
    for c in range(COLS):
        ohB[c % SLOTS, c] = BIG

    in_maps = []
    for c in range(NCORES):
        rows = []
        ind16 = np.zeros((SLOTS, nc_pad), dtype=np.float32)
        ph_col = np.zeros((COLS, 1), dtype=np.float32)
        off = 0
        for s in range(SLOTS):
            gis = np.nonzero(slot_of == c * SLOTS + s)[0]
            if len(gis) == 0:
                continue
            gi = int(gis[0])
            ns = nodes_of[gi]
            rows.extend(ns)
            ind16[s, off:off + len(ns)] = 1
            off += len(ns)
            for h in range(H):
                ph_col[h * SLOTS + s, 0] = phantom[gi]
        emb_c = np.zeros((nc_pad, E), dtype=BF16)
        if rows:
            emb_c[:len(rows)] = graph_emb[np.asarray(rows)].astype(BF16)
        in_maps.append({
            "emb": np.ascontiguousarray(
                emb_c.reshape(nc_pad // 128, 128, E).transpose(1, 0, 2)
                .reshape(128, (nc_pad // 128) * E)),
            "embT": np.ascontiguousarray(
                emb_c.reshape(nc_pad // 512, 512, ES, 128)
                .transpose(0, 3, 2, 1)
                .reshape(nc_pad // 512 * 128, ES * 512)).astype(FP8),
            "mneg": np.ascontiguousarray(ind16 - 1.0).astype(BF16),
            "ohB": ohB,
            "qk": np.ascontiguousarray((np.repeat(qk, SLOTS, axis=0) * QK_SCALE).T.astype(FP8)),
            "ph": ph_col,
            "vT": np.ascontiguousarray(v_w.T).astype(BF16),
            "owT": np.ascontiguousarray(out_w.T).astype(BF16),
            "ob": np.broadcast_to(ob_eff, (SLOTS, D)).astype(np.float32).copy(),
            "ident": np.eye(128, dtype=BF16),
            "ones": np.ones((128, 1), dtype=BF16),
        })

    meta = {
        "bs": bs,
        "slot_of": slot_of,
        "n_nodes": n_nodes,
        "nc_pad": nc_pad,
    }
    return in_maps, meta


def _assemble(results, meta):
    bs = meta["bs"]
    slot_of = meta["slot_of"]
    n_nodes = meta["n_nodes"]
    out = np.empty((bs, D), dtype=np.float32)
    for b in range(bs):
        sl = int(slot_of[b])
        out[b] = results[sl // SLOTS]["out"][sl % SLOTS]
        if n_nodes[b] <= 0:
            out[b] = np.nan
    return out


def kernel(graph_emb, qry, q_w, k_w, v_w, in_b, out_w, out_b, ptr, batch):
    from concourse.bass_utils import run_bass_kernel_spmd

    in_maps, meta = _host_prep(graph_emb, qry, q_w, k_w, v_w, in_b, out_w,
                               out_b, ptr, batch)
    nc_pad = meta["nc_pad"]
    if nc_pad not in _prog_cache:
        _prog_cache[nc_pad] = _build_program(nc_pad)
    nc = _prog_cache[nc_pad]
    res = run_bass_kernel_spmd(nc, in_maps, list(range(NCORES)))
    return _assemble(res.results, meta)
